# revision 1
# baseline (speedup 1.0000x reference)
"""Mamba block on 8 trn2 NeuronCores.

Sharding: data-parallel over batch (2 groups of 4 cores) x tensor-parallel
over d_inner (4-way, 512 channels/core). Device layout is [d_channel, time]
so the selective scan runs as `tensor_tensor_scan` along the free (time)
axis. One 768KB AllReduce of x_dbl per group; out_proj partials are summed
host-side.

ACT engine is restricted to the exp/ln table set, so silu and softplus are
synthesized: softplus(v) = Ln(1+Exp(v)); silu(v) = v*Exp(-Ln(1+Exp(-v))).
"""

import numpy as np

from concourse import bass, mybir, tile
from concourse import bacc
from concourse.bass_utils import run_bass_kernel_spmd

# Model dims (hardcoded; harness runs kernel.py standalone)
D_MODEL = 1024
D_STATE = 16
D_CONV = 4
D_INNER = 2048
DT_RANK = 64
B_SZ, T_LEN = 2, 2048

TP = 4                      # tensor-parallel width within a batch group
DSH = D_INNER // TP         # 512 channels per core
NT = DSH // 128             # 4 d-tiles of 128 channels
NCH = T_LEN // 512          # 4 time chunks of 512 for matmul N-dim
CH = 512

F32 = mybir.dt.float32
F16 = mybir.dt.float16

MUL = mybir.AluOpType.mult
ADD = mybir.AluOpType.add
AF = mybir.ActivationFunctionType


def build_graph():
    nc = bacc.Bacc("TRN2", target_bir_lowering=False, num_devices=8)

    hsT = nc.dram_tensor("hsT", [D_MODEL, T_LEN], F16, kind="ExternalInput")
    w_inT = nc.dram_tensor("w_inT", [D_MODEL, 2 * DSH], F16, kind="ExternalInput")
    w_xT = nc.dram_tensor("w_xT", [DSH, DT_RANK + 2 * D_STATE], F16, kind="ExternalInput")
    w_dtT = nc.dram_tensor("w_dtT", [DT_RANK, DSH], F16, kind="ExternalInput")
    w_outT = nc.dram_tensor("w_outT", [DSH, D_MODEL], F16, kind="ExternalInput")
    conv_w = nc.dram_tensor("conv_w", [NT, 128, D_CONV], F32, kind="ExternalInput")
    # vecs columns: 0=conv_b, 1=b_dt, 2=D, 3=-conv_b
    vecs = nc.dram_tensor("vecs", [NT, 128, 4], F32, kind="ExternalInput")
    a_log = nc.dram_tensor("a_log", [NT, 128, D_STATE], F32, kind="ExternalInput")
    onehot = nc.dram_tensor("onehot", [32, 32 * 128], F16, kind="ExternalInput")
    out_d = nc.dram_tensor("out", [D_MODEL, T_LEN], F32, kind="ExternalOutput")

    with tile.TileContext(nc) as tc:
        with (
            tc.tile_pool(name="wconst", bufs=1) as wconst,
            tc.tile_pool(name="acts", bufs=1) as acts,
            tc.tile_pool(name="outstg", bufs=2) as outstg,
            tc.tile_pool(name="psum", bufs=6, space="PSUM") as psum,
            tc.tile_pool(name="dram", bufs=1, space="DRAM") as dram,
        ):
            # ---- small resident weights/consts ----
            w_x_sb = wconst.tile([128, NT * 96], F16)
            for k in range(NT):
                nc.sync.dma_start(
                    w_x_sb[:, k * 96:(k + 1) * 96],
                    w_xT[k * 128:(k + 1) * 128, :])
            w_dt_sb = wconst.tile([DT_RANK, DSH], F16)
            nc.sync.dma_start(w_dt_sb[:], w_dtT[:])
            w_out_sb = wconst.tile([128, NT * D_MODEL], F16)
            for k in range(NT):
                nc.sync.dma_start(
                    w_out_sb[:, k * D_MODEL:(k + 1) * D_MODEL],
                    w_outT[k * 128:(k + 1) * 128, :])
            conv_w_sb = wconst.tile([128, NT * D_CONV], F32)
            vecs_sb = wconst.tile([128, NT * 4], F32)
            a_log_sb = wconst.tile([128, NT * D_STATE], F32)
            for m in range(NT):
                nc.sync.dma_start(conv_w_sb[:, m * 4:(m + 1) * 4], conv_w[m])
                nc.sync.dma_start(vecs_sb[:, m * 4:(m + 1) * 4], vecs[m])
                nc.sync.dma_start(
                    a_log_sb[:, m * 16:(m + 1) * 16], a_log[m])
            # one-hot selectors: onehot[:, s*128:(s+1)*128] has row s = 1, rest 0
            onehot_sb = wconst.tile([32, 32 * 128], F16)
            nc.sync.dma_start(onehot_sb[:], onehot[:])

            # A = -exp(A_log)
            a_sb = wconst.tile([128, NT * D_STATE], F32)
            nc.scalar.activation(a_sb[:], a_log_sb[:], AF.Exp)
            nc.vector.tensor_scalar_mul(a_sb[:], a_sb[:], -1.0)

            # ---- activations ----
            # x with 3 leading zero cols (conv halo)
            x_all = [acts.tile([128, T_LEN + 3], F16, name=f"x_all{m}") for m in range(NT)]
            z_all = [acts.tile([128, T_LEN], F16, name=f"z_all{m}") for m in range(NT)]
            xc = [acts.tile([128, T_LEN], F16, name=f"xc{m}") for m in range(NT)]
            dt_sb = [acts.tile([128, T_LEN], F16, name=f"dt{m}") for m in range(NT)]
            u_sb = [acts.tile([128, T_LEN], F16, name=f"u{m}") for m in range(NT)]
            y_sb = [acts.tile([128, T_LEN], F16, name=f"y{m}") for m in range(NT)]
            xdbl = acts.tile([96, T_LEN], F32)
            xdbl_f16 = acts.tile([96, T_LEN], F16)
            bc_f16 = acts.tile([32, T_LEN], F16)

            # ---- P1: in_proj (w_in/hs pools freed afterwards) ----
            with (
                tc.tile_pool(name="winp", bufs=1) as winp,
                tc.tile_pool(name="hspool", bufs=1) as hspool,
            ):
                w_in_sb = winp.tile([128, 8 * 1024], F16)
                for k in range(8):
                    nc.sync.dma_start(
                        w_in_sb[:, k * 1024:(k + 1) * 1024],
                        w_inT[k * 128:(k + 1) * 128, :])
                for n in range(NCH):
                    hs_n = hspool.tile([128, 8 * CH], F16, tag="hs")
                    for k in range(8):
                        nc.sync.dma_start(
                            hs_n[:, k * CH:(k + 1) * CH],
                            hsT[k * 128:(k + 1) * 128, n * CH:(n + 1) * CH])
                    for m in range(8):
                        ps = psum.tile([128, CH], F32, tag="ps")
                        for k in range(8):
                            nc.tensor.matmul(
                                ps[:], w_in_sb[:, k * 1024 + m * 128: k * 1024 + (m + 1) * 128],
                                hs_n[:, k * CH:(k + 1) * CH],
                                start=(k == 0), stop=(k == 7))
                        if m < NT:
                            nc.any.tensor_copy(
                                x_all[m][:, 3 + n * CH: 3 + (n + 1) * CH], ps[:])
                        else:
                            nc.any.tensor_copy(
                                z_all[m - NT][:, n * CH:(n + 1) * CH], ps[:])

            # ---- P2: conv + silu(conv+b); gate = silu(z) ----
            # silu(v) = v * sigmoid(v); sigmoid(v) = Exp(-Ln(1 + Exp(-v)))
            with tc.tile_pool(name="convp", bufs=1) as convp:
                for m in range(NT):
                    nc.any.memset(x_all[m][:, 0:3], 0.0)
                    craw = convp.tile([128, T_LEN], F16, tag="craw")
                    ceng = nc.vector
                    ceng.tensor_scalar_mul(
                        craw[:], x_all[m][:, 0:T_LEN], conv_w_sb[:, m * 4: m * 4 + 1])
                    for j in range(1, D_CONV):
                        ceng.scalar_tensor_tensor(
                            craw[:], x_all[m][:, j:j + T_LEN],
                            conv_w_sb[:, m * 4 + j: m * 4 + j + 1], craw[:],
                            op0=MUL, op1=ADD)
                    # v = craw + conv_b (folded); e = exp(-v)
                    t1 = convp.tile([128, T_LEN], F16, tag="t1")
                    nc.scalar.activation(
                        t1[:], craw[:], AF.Exp, scale=-1.0,
                        bias=vecs_sb[:, m * 4 + 3: m * 4 + 4])
                    ceng.tensor_scalar_add(t1[:], t1[:], 1.0)
                    t2 = convp.tile([128, T_LEN], F16, tag="t2")
                    nc.scalar.activation(t2[:], t1[:], AF.Ln)
                    sg = convp.tile([128, T_LEN], F16, tag="sg")
                    nc.scalar.activation(sg[:], t2[:], AF.Exp, scale=-1.0)
                    # xc = (craw + conv_b) * sigmoid
                    ceng.scalar_tensor_tensor(
                        xc[m][:], craw[:], vecs_sb[:, m * 4: m * 4 + 1], sg[:],
                        op0=ADD, op1=MUL)
                    # gate = z * sigmoid(z), in place
                    nc.scalar.activation(t1[:], z_all[m][:], AF.Exp, scale=-1.0)
                    ceng.tensor_scalar_add(t1[:], t1[:], 1.0)
                    nc.scalar.activation(t2[:], t1[:], AF.Ln)
                    nc.scalar.activation(sg[:], t2[:], AF.Exp, scale=-1.0)
                    ceng.tensor_tensor(z_all[m][:], z_all[m][:], sg[:], op=MUL)

            # ---- P3: x_proj partial + AllReduce ----
            for n in range(NCH):
                psx = psum.tile([128, CH], F32, tag="ps")
                for k in range(NT):
                    nc.tensor.matmul(
                        psx[0:96, :], w_x_sb[:, k * 96:(k + 1) * 96],
                        xc[k][:, n * CH:(n + 1) * CH],
                        start=(k == 0), stop=(k == NT - 1))
                nc.any.tensor_copy(xdbl[:, n * CH:(n + 1) * CH], psx[0:96, :])

            cc_in = dram.tile([96, T_LEN], F32)
            cc_out = dram.tile([96, T_LEN], F32)
            nc.sync.dma_start(cc_in[:], xdbl[:])
            nc.gpsimd.collective_compute(
                "AllReduce", ADD,
                replica_groups=[[0, 1, 2, 3], [4, 5, 6, 7]],
                ins=[cc_in.opt()], outs=[cc_out.opt()])
            nc.sync.dma_start(xdbl[:], cc_out[:])
            nc.any.tensor_copy(xdbl_f16[:], xdbl[:])

            with (
                tc.tile_pool(name="bcstg", bufs=2) as bcstg,
                tc.tile_pool(name="sppool", bufs=2) as sppool,
                tc.tile_pool(name="bcast", bufs=2) as bcast,
                tc.tile_pool(name="scantmp", bufs=2) as scantmp,
            ):
                # B/C rows to a partition-0-based tile (f16)
                for n in range(NCH):
                    stg = bcstg.tile([32, CH], F32, tag="bcs")
                    nc.sync.dma_start(stg[:], cc_out[64:96, n * CH:(n + 1) * CH])
                    nc.any.tensor_copy(bc_f16[:, n * CH:(n + 1) * CH], stg[:])

                # ---- P4: dt_proj + softplus = Ln(1+Exp(v+b_dt)) ----
                for m in range(NT):
                    for n in range(NCH):
                        psd = psum.tile([128, CH], F32, tag="ps")
                        nc.tensor.matmul(
                            psd[:], w_dt_sb[:, m * 128:(m + 1) * 128],
                            xdbl_f16[0:DT_RANK, n * CH:(n + 1) * CH],
                            start=True, stop=True)
                        sp = sppool.tile([128, CH], F32, tag="sp")
                        nc.scalar.activation(
                            sp[:], psd[:], AF.Exp,
                            bias=vecs_sb[:, m * 4 + 1: m * 4 + 2])
                        nc.vector.tensor_scalar_add(sp[:], sp[:], 1.0)
                        nc.scalar.activation(
                            dt_sb[m][:, n * CH:(n + 1) * CH], sp[:], AF.Ln)

                # u = dt * conv_x
                for m in range(NT):
                    ueng = nc.vector
                    ueng.tensor_tensor(u_sb[m][:], dt_sb[m][:], xc[m][:], op=MUL)
                    nc.any.memset(y_sb[m][:], 0.0)

                # ---- P6: selective scan over states ----
                for s in range(D_STATE):
                    Bb = bcast.tile([128, T_LEN], F16, tag="Bb")
                    Cb = bcast.tile([128, T_LEN], F16, tag="Cb")
                    for n in range(NCH):
                        psb = psum.tile([128, CH], F32, tag="ps")
                        nc.tensor.matmul(
                            psb[:], onehot_sb[:, s * 128:(s + 1) * 128],
                            bc_f16[:, n * CH:(n + 1) * CH],
                            start=True, stop=True)
                        nc.any.tensor_copy(Bb[:, n * CH:(n + 1) * CH], psb[:])
                        psc = psum.tile([128, CH], F32, tag="ps")
                        nc.tensor.matmul(
                            psc[:], onehot_sb[:, (16 + s) * 128:(17 + s) * 128],
                            bc_f16[:, n * CH:(n + 1) * CH],
                            start=True, stop=True)
                        nc.any.tensor_copy(Cb[:, n * CH:(n + 1) * CH], psc[:])
                    for m in range(NT):
                        eng = nc.vector
                        dA = scantmp.tile([128, T_LEN], F16, tag="dA")
                        nc.scalar.activation(
                            dA[:], dt_sb[m][:], AF.Exp,
                            scale=a_sb[:, m * 16 + s: m * 16 + s + 1])
                        dBx = scantmp.tile([128, T_LEN], F16, tag="dBx")
                        eng.tensor_tensor(dBx[:], u_sb[m][:], Bb[:], op=MUL)
                        h = scantmp.tile([128, T_LEN], F16, tag="h")
                        eng.tensor_tensor_scan(
                            h[:], dA[:], dBx[:], initial=0.0, op0=MUL, op1=ADD)
                        hc = scantmp.tile([128, T_LEN], F16, tag="hc")
                        eng.tensor_tensor(hc[:], h[:], Cb[:], op=MUL)
                        eng.tensor_tensor(y_sb[m][:], y_sb[m][:], hc[:], op=ADD)

                # ---- P7: skip + gate, out_proj ----
                yg = []
                for m in range(NT):
                    ygm = acts.tile([128, T_LEN], F16, name=f"yg{m}")
                    # t = x_conv * D + y ; yg = t * gate
                    geng = nc.vector
                    geng.scalar_tensor_tensor(
                        ygm[:], xc[m][:], vecs_sb[:, m * 4 + 2: m * 4 + 3], y_sb[m][:],
                        op0=MUL, op1=ADD)
                    geng.tensor_tensor(ygm[:], ygm[:], z_all[m][:], op=MUL)
                    yg.append(ygm)

                for mo in range(8):
                    for n in range(NCH):
                        pso = psum.tile([128, CH], F32, tag="ps")
                        for k in range(NT):
                            nc.tensor.matmul(
                                pso[:],
                                w_out_sb[:, k * D_MODEL + mo * 128: k * D_MODEL + (mo + 1) * 128],
                                yg[k][:, n * CH:(n + 1) * CH],
                                start=(k == 0), stop=(k == NT - 1))
                        ot = outstg.tile([128, CH], F32, tag="ot")
                        nc.any.tensor_copy(ot[:], pso[:])
                        nc.sync.dma_start(
                            out_d[mo * 128:(mo + 1) * 128, n * CH:(n + 1) * CH], ot[:])

    nc.finalize()
    return nc


def _onehot():
    oh = np.zeros((32, 32 * 128), np.float16)
    for s in range(32):
        oh[s, s * 128:(s + 1) * 128] = 1.0
    return oh


def make_in_maps(hidden_states, W_in, conv_w, conv_b, W_x, W_dt, b_dt, A_log, D, W_out):
    f16 = np.float16
    in_maps = []
    for core in range(8):
        g, r = divmod(core, TP)
        sh = slice(r * DSH, (r + 1) * DSH)
        m = {
            "hsT": np.ascontiguousarray(hidden_states[g].T).astype(f16),
            "w_inT": np.ascontiguousarray(
                np.concatenate([W_in[sh], W_in[D_INNER + r * DSH: D_INNER + (r + 1) * DSH]], 0).T).astype(f16),
            "w_xT": np.ascontiguousarray(W_x[:, sh].T).astype(f16),
            "w_dtT": np.ascontiguousarray(W_dt[sh].T).astype(f16),
            "w_outT": np.ascontiguousarray(W_out[:, sh].T).astype(f16),
            "conv_w": np.ascontiguousarray(conv_w[sh, 0, :]).reshape(NT, 128, D_CONV).astype(np.float32),
            "vecs": np.stack([conv_b[sh], b_dt[sh], D[sh], -conv_b[sh]], -1).reshape(NT, 128, 4).astype(np.float32),
            "a_log": np.ascontiguousarray(A_log[sh]).reshape(NT, 128, D_STATE).astype(np.float32),
            "onehot": _onehot(),
        }
        in_maps.append(m)
    return in_maps


_NC_CACHE = {}


def kernel(**inputs):
    inputs = {k: np.asarray(v) for k, v in inputs.items()}
    if "nc" not in _NC_CACHE:
        _NC_CACHE["nc"] = build_graph()
    nc = _NC_CACHE["nc"]
    in_maps = make_in_maps(**inputs)
    res = run_bass_kernel_spmd(nc, in_maps, core_ids=list(range(8)))
    outs = res.results
    full = np.zeros((B_SZ, T_LEN, D_MODEL), np.float32)
    for g in range(B_SZ):
        acc = np.zeros((D_MODEL, T_LEN), np.float32)
        for r in range(TP):
            acc += np.asarray(outs[g * TP + r]["out"], np.float32)
        full[g] = acc.T
    return full


if __name__ == "__main__":
    import reference
    ins = reference.setup_inputs()
    ins = {k: np.asarray(v) for k, v in ins.items()}
    exp = np.asarray(reference.reference(**ins))
    got = kernel(**ins)
    err = np.abs(got - exp).max() / (np.abs(exp).max() + 1e-9)
    print("Relative error:", err)

